# revision 17
# baseline (speedup 1.0000x reference)
"""Trainium2 Bass kernel for the batched DSC compute_control problem.

Strategy
--------
The whole reference computation is linear in Y with batch-independent,
lag-indexed coefficient matrices: it folds exactly into

    u[b, :] = sum_{d=1}^{52} W_d @ Y[b, L-d, :]

with W[52, 64, 256] precomputed (float64 on host, cast to fp32) from
M_bar / M / sigma / phi.  On device this is a single GEMM with contraction
K = 52*256 = 13312 over the (lag, p) axis:

    out[c, b] = Wt[K, 64].T @ A[K, 512]         A[(d,p), b] = Y[b, L-d, p]

Sharding: contraction (K) split across the 8 cores — 1664 rows each.  Each
core reads ONLY its A slice (3.4 MB) + its Wt slice (0.43 MB); nothing is
replicated.  Cores produce partial [64, 512] outputs that the host sums —
total device traffic ~4 MB/core, the memory-roofline minimum for this op.

The device program is raw Bass (one sem wait per instruction — the walrus
build here rejects multi-wait instructions, which rules out TileContext's
kernel-tail drain).  Per chunk k of 128 contraction rows, one DMA brings
[W_k | A_k] into SBUF; the PE accumulates 13 matmuls into one PSUM bank;
DVE copies PSUM out; SP DMAs the [64, 512] partial back to DRAM.
"""

import numpy as np

import concourse.bass as bass
import concourse.mybir as mybir
from concourse.bass_utils import run_bass_kernel_spmd

# problem dims (hardcoded per contract)
B, L, P = 512, 128, 256
H, MW, C = 24, 26, 64
D = 2 * MW                # 52 lags
K = D * P                 # 13312 contraction
NCORES = 8
KC = K // NCORES          # 1664 rows per core
NK = KC // 128            # 13 matmul chunks of 128
CH = C + B                # 576 cols per interleaved [W_k | A_k] chunk
MM_DTYPE = mybir.dt.float32r  # PE compute dtype (bit-identical storage to f32)

_CACHE: dict = {}


def _fold_W(M_bar, M, sigma, phi):
    """W[d-1, c, p] s.t. u[b] = sum_d W[d-1] @ Y[b, L-d]; float64 accumulate."""
    h, m = H, MW
    M64 = M.astype(np.float64)
    S = sigma.astype(np.float64) ** 0.25
    ph = phi.astype(np.float64)
    W = np.zeros((D, C, P), np.float64)

    W[0] += M_bar.astype(np.float64)

    J1 = min(m, L - 1, h + 1)
    coef1 = S[:h, None] * ph[:J1, :h].T
    W[:J1] += np.einsum("ij,ijcp->jcp", coef1, M64[1 : h + 1, :J1])

    K2 = min(m + 1, L)
    Mk = np.einsum("l,kl,lcp->kcp", S, ph[:K2], M64[0])
    for k in range(K2):
        W[max(k, 1) - 1] += Mk[k]

    Sprod = (sigma[:h, None].astype(np.float64) * sigma[None, :].astype(np.float64)) ** 0.25
    A3 = np.zeros((D, h, h + 1), np.float64)
    for jp in range(m):
        for k in range(m + 1):
            A3[jp + k] += np.outer(ph[jp, :h], ph[k, : h + 1])
    C3 = A3 * Sprod[None]
    W += (C3.reshape(D, -1) @ M64[1 : h + 1].reshape(-1, C * P)).reshape(D, C, P)
    return W


# chunk groups: two HWDGE rings (SP, ACT) stream one group at a time.
# Matmuls consume groups in expected-arrival order (ring-interleaved).
GROUPS = [(0, 3), (3, 6), (6, 9), (9, 12), (12, NK)]  # chunk ranges
RING = [0, 0, 1, 1, 1]                           # 0 = SP ring, 1 = ACT ring
MM_ORDER = [0, 2, 1, 3, 4]                       # group consumption order
# last-consumed group is a single chunk so the matmul tail after the final
# DMA is one LDW+MM (~0.7us) instead of a whole group


def _build_program():
    """Raw-Bass SPMD program: 13-chunk K-accumulated GEMM -> partial [C, B]."""
    nc = bass.Bass()
    f32 = mybir.dt.float32
    aw_ins = [
        nc.declare_dram_parameter(
            f"AW{g}", [128, (hi - lo) * CH], MM_DTYPE, isOutput=False
        )
        for g, (lo, hi) in enumerate(GROUPS)
    ]
    out_d = nc.declare_dram_parameter("out", [C, B], f32, isOutput=True)

    import contextlib

    with contextlib.ExitStack() as ctx:
        aw = ctx.enter_context(nc.sbuf_tensor([128, NK * CH], MM_DTYPE))
        o_tile = ctx.enter_context(nc.sbuf_tensor([C, B], f32))
        ps = ctx.enter_context(nc.psum_tensor([C, B], f32))
        dma_sems = [ctx.enter_context(nc.semaphore(f"dma{g}")) for g in range(len(GROUPS))]
        pe_sem = ctx.enter_context(nc.semaphore("pe"))
        dve_sem = ctx.enter_context(nc.semaphore("dve"))
        out_sem = ctx.enter_context(nc.semaphore("outd"))
        block = ctx.enter_context(nc.Block())

        @block.sync
        def _(sync):
            for g, (lo, hi) in enumerate(GROUPS):
                if RING[g] == 0:
                    sync.dma_start(
                        aw[:, lo * CH : hi * CH], aw_ins[g][:]
                    ).then_inc(dma_sems[g], 16)
            sync.wait_ge(dve_sem, 1)
            sync.dma_start(out_d[:], o_tile[:]).then_inc(out_sem, 16)
            sync.wait_ge(out_sem, 16)

        @block.scalar
        def _(scalar):
            for g, (lo, hi) in enumerate(GROUPS):
                if RING[g] == 1:
                    scalar.dma_start(
                        aw[:, lo * CH : hi * CH], aw_ins[g][:]
                    ).then_inc(dma_sems[g], 16)

        @block.tensor
        def _(tensor):
            first, last = MM_ORDER[0], MM_ORDER[-1]
            for g in MM_ORDER:
                lo, hi = GROUPS[g]
                tensor.wait_ge(dma_sems[g], 16)
                for k in range(lo, hi):
                    mm = tensor.matmul(
                        ps[:],
                        aw[:, k * CH : k * CH + C],
                        aw[:, k * CH + C : (k + 1) * CH],
                        start=(g == first and k == lo),
                        stop=(g == last and k == hi - 1),
                    )
            mm.then_inc(pe_sem, 1)

        @block.vector
        def _(vector):
            vector.wait_ge(pe_sem, 1)
            vector.tensor_copy(o_tile[:], ps[:]).then_inc(dve_sem, 1)

    return nc


def _get_program():
    if "nc" not in _CACHE:
        _CACHE["nc"] = _build_program()
    return _CACHE["nc"]


def _shard_rows(X):
    """[K, N] row-sharded -> per-core [128, NK, N] in SBUF-DMA layout."""
    n = X.shape[1]
    shards = []
    for i in range(NCORES):
        s = X[i * KC : (i + 1) * KC]                       # [KC, n]
        shards.append(s.reshape(NK, 128, n).transpose(1, 0, 2))  # [128, NK, n]
    return shards


def kernel(Y, M_bar, M, sigma, phi):
    Y = np.asarray(Y, np.float32)
    W = _fold_W(np.asarray(M_bar), np.asarray(M), np.asarray(sigma), np.asarray(phi))
    # Wt[(d,p), c]
    Wt = np.ascontiguousarray(W.transpose(0, 2, 1).reshape(K, C), np.float32)
    # A[(d,p), b] = Y[b, L-d, p]
    A = np.ascontiguousarray(
        Y[:, L - D :, :][:, ::-1, :].transpose(1, 2, 0).reshape(K, B)
    )

    a_shards = _shard_rows(A)   # [128, NK, B]
    w_shards = _shard_rows(Wt)  # [128, NK, C]
    in_maps = []
    for i in range(NCORES):
        aw = np.concatenate([w_shards[i], a_shards[i]], axis=2)  # [128, NK, CH]
        m = {}
        for g, (lo, hi) in enumerate(GROUPS):
            m[f"AW{g}"] = np.ascontiguousarray(
                aw[:, lo:hi].reshape(128, (hi - lo) * CH)
            )
        in_maps.append(m)

    nc = _get_program()
    res = run_bass_kernel_spmd(
        nc, in_maps, list(range(NCORES)), **_CACHE.get("run_kwargs", {})
    )
    _CACHE["last_result"] = res
    partial = np.zeros((C, B), np.float64)
    for r in res.results:
        partial += r["out"].astype(np.float64)
    return np.ascontiguousarray(partial.T, np.float32)


# revision 21
# speedup vs baseline: 1.3152x; 1.3152x over previous
"""Trainium2 Bass kernel for the batched DSC compute_control problem.

Strategy
--------
The whole reference computation is linear in Y with batch-independent,
lag-indexed coefficient matrices: it folds exactly into

    u[b, :] = sum_{d=1}^{52} W_d @ Y[b, L-d, :]

with W[52, 64, 256] precomputed (float64 on host, cast to fp32) from
M_bar / M / sigma / phi.  On device this is a single GEMM with contraction
K = 52*256 = 13312 over the (lag, p) axis:

    out[c, b] = Wt[K, 64].T @ A[K, 512]         A[(d,p), b] = Y[b, L-d, p]

Sharding: contraction (K) split across the 8 cores — 1664 rows each.  Each
core reads ONLY its A slice (3.4 MB) + its Wt slice (0.43 MB); nothing is
replicated.  Cores produce partial [64, 512] outputs that the host sums —
total device traffic ~4 MB/core, the memory-roofline minimum for this op.

The device program is raw Bass (one sem wait per instruction — the walrus
build here rejects multi-wait instructions, which rules out TileContext's
kernel-tail drain).  Per chunk k of 128 contraction rows, one DMA brings
[W_k | A_k] into SBUF; the PE accumulates 13 matmuls into one PSUM bank;
DVE copies PSUM out; SP DMAs the [64, 512] partial back to DRAM.
"""

import numpy as np

import concourse.bass as bass
import concourse.mybir as mybir
from concourse.bass_utils import run_bass_kernel_spmd

# problem dims (hardcoded per contract)
B, L, P = 512, 128, 256
H, MW, C = 24, 26, 64
D = 2 * MW                # 52 lags
K = D * P                 # 13312 contraction
NCORES = 8
KC = K // NCORES          # 1664 rows per core
NK = KC // 128            # 13 matmul chunks of 128
CH = C + B                # 576 cols per interleaved [W_k | A_k] chunk
# PE compute/storage dtype for the streamed operands.  float16 halves the DMA
# traffic (the kernel is memory-bound) and keeps 10 mantissa bits: measured
# end-to-end rel err ~4e-4 vs the fp32 reference (float32r: ~1.5e-4 at 2x the
# bytes; PSUM accumulation is fp32 either way).
MM_DTYPE = mybir.dt.float16
NP_IN = np.float16

_CACHE: dict = {}


def _fold_W(M_bar, M, sigma, phi):
    """W[d-1, c, p] s.t. u[b] = sum_d W[d-1] @ Y[b, L-d]; float64 accumulate."""
    h, m = H, MW
    M64 = M.astype(np.float64)
    S = sigma.astype(np.float64) ** 0.25
    ph = phi.astype(np.float64)
    W = np.zeros((D, C, P), np.float64)

    W[0] += M_bar.astype(np.float64)

    J1 = min(m, L - 1, h + 1)
    coef1 = S[:h, None] * ph[:J1, :h].T
    W[:J1] += np.einsum("ij,ijcp->jcp", coef1, M64[1 : h + 1, :J1])

    K2 = min(m + 1, L)
    Mk = np.einsum("l,kl,lcp->kcp", S, ph[:K2], M64[0])
    for k in range(K2):
        W[max(k, 1) - 1] += Mk[k]

    Sprod = (sigma[:h, None].astype(np.float64) * sigma[None, :].astype(np.float64)) ** 0.25
    A3 = np.zeros((D, h, h + 1), np.float64)
    for jp in range(m):
        for k in range(m + 1):
            A3[jp + k] += np.outer(ph[jp, :h], ph[k, : h + 1])
    C3 = A3 * Sprod[None]
    W += (C3.reshape(D, -1) @ M64[1 : h + 1].reshape(-1, C * P)).reshape(D, C, P)
    return W


# chunk groups: two HWDGE rings (SP, ACT) stream one group at a time.
# Matmuls consume groups in expected-arrival order (ring-interleaved).
GROUPS = [(0, 3), (3, 6), (6, 9), (9, 11), (11, NK)]  # chunk ranges
RING = [0, 0, 1, 1, 2]                  # 0 = SP ring, 1 = ACT ring, 2 = SWDGE
MM_ORDER = [0, 2, 1, 3, 4]              # group consumption order
# three DMA issuers (SP + ACT HWDGE rings, GpSimd SWDGE) stream in parallel;
# last-consumed groups are small so the matmul tail after the final DMA
# arrival is short


def _build_program():
    """Raw-Bass SPMD program: 13-chunk K-accumulated GEMM -> partial [C, B]."""
    nc = bass.Bass()
    f32 = mybir.dt.float32
    aw_ins = [
        nc.declare_dram_parameter(
            f"AW{g}", [128, (hi - lo) * CH], MM_DTYPE, isOutput=False
        )
        for g, (lo, hi) in enumerate(GROUPS)
    ]
    out_d = nc.declare_dram_parameter("out", [C, B], f32, isOutput=True)

    import contextlib

    with contextlib.ExitStack() as ctx:
        aw = ctx.enter_context(nc.sbuf_tensor([128, NK * CH], MM_DTYPE))
        o_tile = ctx.enter_context(nc.sbuf_tensor([C, B], f32))
        ps = ctx.enter_context(nc.psum_tensor([C, B], f32))
        dma_sems = [ctx.enter_context(nc.semaphore(f"dma{g}")) for g in range(len(GROUPS))]
        pe_sem = ctx.enter_context(nc.semaphore("pe"))
        dve_sem = ctx.enter_context(nc.semaphore("dve"))
        out_sem = ctx.enter_context(nc.semaphore("outd"))
        out2_sem = ctx.enter_context(nc.semaphore("outd2"))
        block = ctx.enter_context(nc.Block())

        HB = B // 2  # output halves, one per HWDGE ring

        @block.sync
        def _(sync):
            for g, (lo, hi) in enumerate(GROUPS):
                if RING[g] == 0:
                    sync.dma_start(
                        aw[:, lo * CH : hi * CH], aw_ins[g][:]
                    ).then_inc(dma_sems[g], 16)
            sync.wait_ge(dve_sem, 1)
            sync.dma_start(out_d[:, :HB], o_tile[:, :HB]).then_inc(out_sem, 16)
            sync.wait_ge(out_sem, 16)

        @block.scalar
        def _(scalar):
            for g, (lo, hi) in enumerate(GROUPS):
                if RING[g] == 1:
                    scalar.dma_start(
                        aw[:, lo * CH : hi * CH], aw_ins[g][:]
                    ).then_inc(dma_sems[g], 16)
            scalar.wait_ge(dve_sem, 2)
            scalar.dma_start(out_d[:, HB:], o_tile[:, HB:]).then_inc(out2_sem, 16)
            scalar.wait_ge(out2_sem, 16)

        @block.gpsimd
        def _(gpsimd):
            for g, (lo, hi) in enumerate(GROUPS):
                if RING[g] == 2:
                    gpsimd.dma_start(
                        aw[:, lo * CH : hi * CH], aw_ins[g][:]
                    ).then_inc(dma_sems[g], 16)

        @block.tensor
        def _(tensor):
            first, last = MM_ORDER[0], MM_ORDER[-1]
            for g in MM_ORDER:
                lo, hi = GROUPS[g]
                tensor.wait_ge(dma_sems[g], 16)
                for k in range(lo, hi):
                    mm = tensor.matmul(
                        ps[:],
                        aw[:, k * CH : k * CH + C],
                        aw[:, k * CH + C : (k + 1) * CH],
                        start=(g == first and k == lo),
                        stop=(g == last and k == hi - 1),
                    )
            mm.then_inc(pe_sem, 1)

        @block.vector
        def _(vector):
            vector.wait_ge(pe_sem, 1)
            vector.tensor_copy(o_tile[:, :HB], ps[:, :HB]).then_inc(dve_sem, 1)
            vector.tensor_copy(o_tile[:, HB:], ps[:, HB:]).then_inc(dve_sem, 1)

    return nc


def _get_program():
    if "nc" not in _CACHE:
        _CACHE["nc"] = _build_program()
    return _CACHE["nc"]


def _shard_rows(X):
    """[K, N] row-sharded -> per-core [128, NK, N] in SBUF-DMA layout."""
    n = X.shape[1]
    shards = []
    for i in range(NCORES):
        s = X[i * KC : (i + 1) * KC]                       # [KC, n]
        shards.append(s.reshape(NK, 128, n).transpose(1, 0, 2))  # [128, NK, n]
    return shards


def kernel(Y, M_bar, M, sigma, phi):
    Y = np.asarray(Y, np.float32)
    W = _fold_W(np.asarray(M_bar), np.asarray(M), np.asarray(sigma), np.asarray(phi))
    # Wt[(d,p), c]
    Wt = np.ascontiguousarray(W.transpose(0, 2, 1).reshape(K, C), NP_IN)
    # A[(d,p), b] = Y[b, L-d, p]
    A = np.ascontiguousarray(
        Y[:, L - D :, :][:, ::-1, :].transpose(1, 2, 0).reshape(K, B), NP_IN
    )

    a_shards = _shard_rows(A)   # [128, NK, B]
    w_shards = _shard_rows(Wt)  # [128, NK, C]
    in_maps = []
    for i in range(NCORES):
        aw = np.concatenate([w_shards[i], a_shards[i]], axis=2)  # [128, NK, CH]
        m = {}
        for g, (lo, hi) in enumerate(GROUPS):
            m[f"AW{g}"] = np.ascontiguousarray(
                aw[:, lo:hi].reshape(128, (hi - lo) * CH)
            )
        in_maps.append(m)

    nc = _get_program()
    res = run_bass_kernel_spmd(
        nc, in_maps, list(range(NCORES)), **_CACHE.get("run_kwargs", {})
    )
    _CACHE["last_result"] = res
    partial = np.zeros((C, B), np.float64)
    for r in res.results:
        partial += r["out"].astype(np.float64)
    return np.ascontiguousarray(partial.T, np.float32)


# revision 24
# speedup vs baseline: 1.3733x; 1.0442x over previous
"""Trainium2 Bass kernel for the batched DSC compute_control problem.

Strategy
--------
The whole reference computation is linear in Y with batch-independent,
lag-indexed coefficient matrices: it folds exactly into

    u[b, :] = sum_{d=1}^{52} W_d @ Y[b, L-d, :]

with W[52, 64, 256] precomputed (float64 on host, cast to fp32) from
M_bar / M / sigma / phi.  On device this is a single GEMM with contraction
K = 52*256 = 13312 over the (lag, p) axis:

    out[c, b] = Wt[K, 64].T @ A[K, 512]         A[(d,p), b] = Y[b, L-d, p]

Sharding: contraction (K) split across the 8 cores — 1664 rows each.  Each
core reads ONLY its A slice (3.4 MB) + its Wt slice (0.43 MB); nothing is
replicated.  Cores produce partial [64, 512] outputs that the host sums —
total device traffic ~4 MB/core, the memory-roofline minimum for this op.

The device program is raw Bass (one sem wait per instruction — the walrus
build here rejects multi-wait instructions, which rules out TileContext's
kernel-tail drain).  Per chunk k of 128 contraction rows, one DMA brings
[W_k | A_k] into SBUF; the PE accumulates 13 matmuls into one PSUM bank;
DVE copies PSUM out; SP DMAs the [64, 512] partial back to DRAM.
"""

import numpy as np

import concourse.bass as bass
import concourse.mybir as mybir
from concourse.bass_utils import run_bass_kernel_spmd

# problem dims (hardcoded per contract)
B, L, P = 512, 128, 256
H, MW, C = 24, 26, 64
D = 2 * MW                # 52 lags
K = D * P                 # 13312 contraction
NCORES = 8
KC = K // NCORES          # 1664 rows per core
NK = KC // 128            # 13 matmul chunks of 128
CH = C + B                # 576 cols per interleaved [W_k | A_k] chunk
# PE compute/storage dtype for the streamed operands.  float16 halves the DMA
# traffic (the kernel is memory-bound) and keeps 10 mantissa bits: measured
# end-to-end rel err ~4e-4 vs the fp32 reference (float32r: ~1.5e-4 at 2x the
# bytes; PSUM accumulation is fp32 either way).
MM_DTYPE = mybir.dt.float16
NP_IN = np.float16

_CACHE: dict = {}


def _fold_W(M_bar, M, sigma, phi):
    """W[d-1, c, p] s.t. u[b] = sum_d W[d-1] @ Y[b, L-d]; float64 accumulate."""
    h, m = H, MW
    M64 = M.astype(np.float64)
    S = sigma.astype(np.float64) ** 0.25
    ph = phi.astype(np.float64)
    W = np.zeros((D, C, P), np.float64)

    W[0] += M_bar.astype(np.float64)

    J1 = min(m, L - 1, h + 1)
    coef1 = S[:h, None] * ph[:J1, :h].T
    W[:J1] += np.einsum("ij,ijcp->jcp", coef1, M64[1 : h + 1, :J1])

    K2 = min(m + 1, L)
    Mk = np.einsum("l,kl,lcp->kcp", S, ph[:K2], M64[0])
    for k in range(K2):
        W[max(k, 1) - 1] += Mk[k]

    Sprod = (sigma[:h, None].astype(np.float64) * sigma[None, :].astype(np.float64)) ** 0.25
    A3 = np.zeros((D, h, h + 1), np.float64)
    for jp in range(m):
        for k in range(m + 1):
            A3[jp + k] += np.outer(ph[jp, :h], ph[k, : h + 1])
    C3 = A3 * Sprod[None]
    W += (C3.reshape(D, -1) @ M64[1 : h + 1].reshape(-1, C * P)).reshape(D, C, P)
    return W


# chunk groups: two HWDGE rings (SP, ACT) stream one group at a time.
# Matmuls consume groups in expected-arrival order (ring-interleaved).
GROUPS = [(0, 3), (3, 6), (6, 9), (9, 12), (12, NK)]  # chunk ranges
RING = [0, 0, 1, 1, 1]                  # 0 = SP ring, 1 = ACT ring
MM_ORDER = [0, 2, 1, 3, 4]              # group consumption order
# two HWDGE rings stream groups in parallel; the last-consumed group is a
# single chunk so the matmul tail after the final DMA arrival is one LDW+MM


def _build_program():
    """Raw-Bass SPMD program: 13-chunk K-accumulated GEMM -> partial [C, B]."""
    nc = bass.Bass()
    f32 = mybir.dt.float32
    aw_ins = [
        nc.declare_dram_parameter(
            f"AW{g}", [128, (hi - lo) * CH], MM_DTYPE, isOutput=False
        )
        for g, (lo, hi) in enumerate(GROUPS)
    ]
    out_d = nc.declare_dram_parameter("out", [C, B], f32, isOutput=True)

    import contextlib

    with contextlib.ExitStack() as ctx:
        aw = ctx.enter_context(nc.sbuf_tensor([128, NK * CH], MM_DTYPE))
        o_tile = ctx.enter_context(nc.sbuf_tensor([C, B], f32))
        ps = ctx.enter_context(nc.psum_tensor([C, B], f32))
        dma_sems = [ctx.enter_context(nc.semaphore(f"dma{g}")) for g in range(len(GROUPS))]
        pe_sem = ctx.enter_context(nc.semaphore("pe"))
        dve_sem = ctx.enter_context(nc.semaphore("dve"))
        out_sem = ctx.enter_context(nc.semaphore("outd"))
        block = ctx.enter_context(nc.Block())

        @block.sync
        def _(sync):
            for g, (lo, hi) in enumerate(GROUPS):
                if RING[g] == 0:
                    sync.dma_start(
                        aw[:, lo * CH : hi * CH], aw_ins[g][:]
                    ).then_inc(dma_sems[g], 16)
            sync.wait_ge(dve_sem, 1)
            sync.dma_start(out_d[:], o_tile[:]).then_inc(out_sem, 16)
            sync.wait_ge(out_sem, 16)

        @block.scalar
        def _(scalar):
            for g, (lo, hi) in enumerate(GROUPS):
                if RING[g] == 1:
                    scalar.dma_start(
                        aw[:, lo * CH : hi * CH], aw_ins[g][:]
                    ).then_inc(dma_sems[g], 16)

        @block.tensor
        def _(tensor):
            first, last = MM_ORDER[0], MM_ORDER[-1]
            for g in MM_ORDER:
                lo, hi = GROUPS[g]
                tensor.wait_ge(dma_sems[g], 16)
                for k in range(lo, hi):
                    mm = tensor.matmul(
                        ps[:],
                        aw[:, k * CH : k * CH + C],
                        aw[:, k * CH + C : (k + 1) * CH],
                        start=(g == first and k == lo),
                        stop=(g == last and k == hi - 1),
                    )
            mm.then_inc(pe_sem, 1)

        @block.vector
        def _(vector):
            vector.wait_ge(pe_sem, 1)
            vector.tensor_copy(o_tile[:], ps[:]).then_inc(dve_sem, 1)

    return nc


def _get_program():
    if "nc" not in _CACHE:
        _CACHE["nc"] = _build_program()
    return _CACHE["nc"]


def _shard_rows(X):
    """[K, N] row-sharded -> per-core [128, NK, N] in SBUF-DMA layout."""
    n = X.shape[1]
    shards = []
    for i in range(NCORES):
        s = X[i * KC : (i + 1) * KC]                       # [KC, n]
        shards.append(s.reshape(NK, 128, n).transpose(1, 0, 2))  # [128, NK, n]
    return shards


def kernel(Y, M_bar, M, sigma, phi):
    Y = np.asarray(Y, np.float32)
    W = _fold_W(np.asarray(M_bar), np.asarray(M), np.asarray(sigma), np.asarray(phi))
    # Wt[(d,p), c]
    Wt = np.ascontiguousarray(W.transpose(0, 2, 1).reshape(K, C), NP_IN)
    # A[(d,p), b] = Y[b, L-d, p]
    A = np.ascontiguousarray(
        Y[:, L - D :, :][:, ::-1, :].transpose(1, 2, 0).reshape(K, B), NP_IN
    )

    a_shards = _shard_rows(A)   # [128, NK, B]
    w_shards = _shard_rows(Wt)  # [128, NK, C]
    in_maps = []
    for i in range(NCORES):
        aw = np.concatenate([w_shards[i], a_shards[i]], axis=2)  # [128, NK, CH]
        m = {}
        for g, (lo, hi) in enumerate(GROUPS):
            m[f"AW{g}"] = np.ascontiguousarray(
                aw[:, lo:hi].reshape(128, (hi - lo) * CH)
            )
        in_maps.append(m)

    nc = _get_program()
    res = run_bass_kernel_spmd(
        nc, in_maps, list(range(NCORES)), **_CACHE.get("run_kwargs", {})
    )
    _CACHE["last_result"] = res
    partial = np.zeros((C, B), np.float64)
    for r in res.results:
        partial += r["out"].astype(np.float64)
    return np.ascontiguousarray(partial.T, np.float32)
